# revision 8
# baseline (speedup 1.0000x reference)
"""CapsuleLayer dynamic-routing kernel for 8 Trainium2 NeuronCores.

Math (reference):
    u_hat[b,n,j,d] = sum_i W[n,j,d,i] * x[b,j,i]
    b = 0; for r in 0..2:
        c = softmax_n(b); s[b,n,d] = sum_j c*u_hat; v = squash_d(s)
        if r < 2: b += sum_d v*u_hat
    return v  [B, N, D]

Sharding: J (input capsules, 2048) split 8 ways -> Jc=256 per core.
Softmax over n is local; only s needs a 256 KiB AllReduce per iteration.

Per-core dataflow, one sweep over W per routing iteration (u_hat is
recomputed from SBUF-streamed W each iteration; never materialized):
  - j processed in groups of 4: 4 PE sub-matmuls via column tiling
    (tile_position=(0,32r)) produce u_hat group tile
    [128=(4j x 32b), (n,d)] in PSUM.
  - r0: softmax(0) is uniform, so u_hat is accumulated over all j
    directly in PSUM; s0 = (1/N) * strip-sum. No vector work at all.
  - r>=1: logits[p=(j,b), (g,n)] += sum_d v_{r-1}*u_hat  (DVE mult +
    segmented reduce over d); softmax over n is local to each
    (partition, group) -> c; tmp2 = c (x) u_hat on GpSimd.
  - s accumulated over j by a PE matmul with a stacked-identity lhsT
    (sums the 4 j-strips per b), accumulating across groups in PSUM.
    s-matmuls are emitted one group late so they don't block the next
    group's u_hat matmuls in the in-order PE queue.
  - AllReduce s across cores, squash redundantly on every core.
"""

import functools
import numpy as np

B, J, I = 32, 2048, 16
N, D = 64, 32
NCORES = 8
JC = J // NCORES          # 256 j per core
GRP = 4                   # j's per group (PE column strips)
NG = JC // GRP            # 64 groups
ND = N * D                # 2048
HALF = ND // 2            # 1024 free-dim half (PSUM sizing)
NH = N // 2               # 32 n per half
ROUTINGS = 3
EPS = 1e-7


@functools.lru_cache(maxsize=1)
def _build():
    import concourse.bass as bass
    import concourse.mybir as mybir
    import concourse.bacc as bacc
    import concourse.tile as tile

    f32 = mybir.dt.float32
    bf16 = mybir.dt.bfloat16
    MUL = mybir.AluOpType.mult
    ADD = mybir.AluOpType.add
    AX = mybir.AxisListType.X
    AF = mybir.ActivationFunctionType

    nc = bacc.Bacc("TRN2", target_bir_lowering=False, debug=False,
                   num_devices=NCORES)

    xt_d = nc.dram_tensor("xt", [I, JC * B], bf16, kind="ExternalInput")
    wt_d = nc.dram_tensor("wt", [I, JC, ND], bf16, kind="ExternalInput")
    ones_d = nc.dram_tensor("ones4", [GRP * B, B], bf16, kind="ExternalInput")
    v_d = nc.dram_tensor("v", [B, ND], f32, kind="ExternalOutput")

    with tile.TileContext(nc) as tc:
        with (
            tc.tile_pool(name="persist", bufs=1) as pp,
            tc.tile_pool(name="wstream", bufs=3) as wp,
            tc.tile_pool(name="work", bufs=3) as wk,
            tc.tile_pool(name="small", bufs=2) as sm,
            tc.tile_pool(name="ups", bufs=2, space="PSUM") as ups_pool,
            tc.tile_pool(name="sps", bufs=1, space="PSUM") as sps_pool,
            tc.tile_pool(name="dram", bufs=1, space="DRAM") as dr,
        ):
            xt = pp.tile([I, JC * B], bf16)
            nc.sync.dma_start(xt[:], xt_d[:])
            ones4 = pp.tile([GRP * B, B], bf16)
            nc.sync.dma_start(ones4[:], ones_d[:])

            logits = pp.tile([128, NG, N], f32)
            v_rep = pp.tile([128, N, D], bf16)
            v_small = pp.tile([B, ND], bf16)
            s_sb = pp.tile([B, N, D], f32)
            v_sb = pp.tile([B, ND], f32)

            cc_in = dr.tile([B, ND], f32)
            cc_out = dr.tile([B, ND], f32)

            def u_mms(u_ps, w_t, g, h, start, stop):
                """16 col-tiled matmuls for one (group, half); rr-outer so
                consecutive chunk matmuls share the stationary lhsT."""
                for rr in range(GRP):
                    j = g * GRP + rr
                    for cch in range(2):
                        nc.tensor.matmul(
                            u_ps[32 * rr:32 * rr + 32,
                                 cch * 512:(cch + 1) * 512],
                            xt[:, j * B:(j + 1) * B],
                            w_t[:, rr, h * HALF + cch * 512:
                                h * HALF + (cch + 1) * 512],
                            start=start, stop=stop,
                            tile_position=(0, 32 * rr),
                            skip_group_check=True,
                        )

            for r in range(ROUTINGS):
                s_ps = sps_pool.tile([B, ND], f32)

                if r == 0:
                    # -- r0: c is uniform; accumulate u_hat over j in PSUM --
                    acc = [ups_pool.tile([128, HALF], f32, name=f"acc{_h}", tag="u_ps") for _h in range(2)]
                    for g in range(NG):
                        w_t = wp.tile([I, GRP, ND], bf16)
                        nc.sync.dma_start(
                            w_t[:], wt_d[:, g * GRP:(g + 1) * GRP, :])
                        for h in range(2):
                            u_mms(acc[h], w_t, g, h,
                                  start=(g == 0), stop=(g == NG - 1))
                    # evac to bf16 SBUF, then strip-sum via ones4 matmul
                    for h in range(2):
                        a_sb = wk.tile([128, HALF], bf16)
                        nc.scalar.activation(a_sb[:], acc[h][:], AF.Copy)
                        for cch in range(2):
                            nc.tensor.matmul(
                                s_ps[:, h * HALF + cch * 512:
                                     h * HALF + (cch + 1) * 512],
                                ones4[:],
                                a_sb[:, cch * 512:(cch + 1) * 512],
                                start=True, stop=True,
                                skip_group_check=True,
                            )
                else:
                    # -- r>=1: fused logits update + local softmax + s --
                    pending_smm = []
                    for g in range(NG):
                        w_t = wp.tile([I, GRP, ND], bf16)
                        nc.sync.dma_start(
                            w_t[:], wt_d[:, g * GRP:(g + 1) * GRP, :])

                        c_t = sm.tile([128, N], bf16)
                        zrec = sm.tile([128, 1], f32)

                        u_sb_halves = []
                        for h in range(2):
                            u_ps = ups_pool.tile([128, HALF], f32)
                            u_mms(u_ps, w_t, g, h, start=True, stop=True)
                            # emit previous group's s-matmuls behind this
                            # group's u-matmuls in the PE stream
                            if pending_smm:
                                pending_smm.pop(0)()

                            u_sb = wk.tile([128, NH, D], bf16)
                            nc.scalar.activation(u_sb[:], u_ps[:], AF.Copy)
                            tl = wk.tile([128, NH, D], bf16)
                            nc.vector.tensor_tensor(
                                tl[:], u_sb[:],
                                v_rep[:, h * NH:(h + 1) * NH, :], op=MUL)
                            if r == 1:
                                nc.vector.tensor_reduce(
                                    logits[:, g, h * NH:(h + 1) * NH],
                                    tl[:], axis=AX, op=ADD)
                            else:
                                dtmp = sm.tile([128, NH], f32)
                                nc.vector.tensor_reduce(
                                    dtmp[:], tl[:], axis=AX, op=ADD)
                                nc.vector.tensor_add(
                                    logits[:, g, h * NH:(h + 1) * NH],
                                    logits[:, g, h * NH:(h + 1) * NH],
                                    dtmp[:])
                            u_sb_halves.append(u_sb)

                        # local softmax over n for this group's 4 j's
                        e_t = sm.tile([128, N], f32)
                        nc.scalar.activation(e_t[:], logits[:, g, :], AF.Exp)
                        zsum = sm.tile([128, 1], f32)
                        nc.vector.tensor_reduce(zsum[:], e_t[:], axis=AX, op=ADD)
                        nc.vector.reciprocal(zrec[:], zsum[:])
                        nc.vector.tensor_scalar_mul(c_t[:], e_t[:], zrec[:])

                        # tmp2 = c (x) u_hat on GpSimd (c broadcast over d)
                        t2s = []
                        for h in range(2):
                            t2 = wk.tile([128, NH, D], bf16, name="t2", tag="t2")
                            nc.gpsimd.tensor_tensor(
                                t2[:], u_sb_halves[h][:],
                                c_t[:, h * NH:(h + 1) * NH, None]
                                .broadcast_to([128, NH, D]),
                                op=MUL)
                            t2s.append(t2)

                        def make_smm(t2s=t2s, g=g):
                            def emit():
                                for h in range(2):
                                    t2f = t2s[h][:].rearrange("p a b -> p (a b)")
                                    for cch in range(2):
                                        nc.tensor.matmul(
                                            s_ps[:, h * HALF + cch * 512:
                                                 h * HALF + (cch + 1) * 512],
                                            ones4[:],
                                            t2f[:, cch * 512:(cch + 1) * 512],
                                            start=(g == 0), stop=(g == NG - 1),
                                            skip_group_check=True,
                                        )
                            return emit
                        pending_smm.append(make_smm())
                    while pending_smm:
                        pending_smm.pop(0)()

                # ---- end of sweep: AllReduce s, squash, update v ----
                s_evac = sm.tile([B, ND], f32)
                if r == 0:
                    nc.vector.tensor_scalar_mul(s_evac[:], s_ps[:], 1.0 / N)
                else:
                    nc.vector.tensor_copy(s_evac[:], s_ps[:])
                nc.sync.dma_start(cc_in[:], s_evac[:])
                nc.gpsimd.collective_compute(
                    "AllReduce", ADD,
                    replica_groups=[list(range(NCORES))],
                    ins=[cc_in[:].opt()], outs=[cc_out[:].opt()],
                )
                nc.sync.dma_start(s_sb[:], cc_out[:])

                sq = wk.tile([B, N, D], f32)
                nc.vector.tensor_tensor(sq[:], s_sb[:], s_sb[:], op=MUL)
                ns2 = sm.tile([B, N], f32)
                nc.vector.tensor_reduce(ns2[:], sq[:], axis=AX, op=ADD)
                onep = sm.tile([B, N], f32)
                nc.vector.tensor_scalar_add(onep[:], ns2[:], 1.0)
                rt = sm.tile([B, N], f32)
                eps_t = sm.tile([B, 1], f32)
                nc.vector.memset(eps_t[:], EPS)
                nc.scalar.activation(rt[:], ns2[:], AF.Sqrt, bias=eps_t[:])
                den = sm.tile([B, N], f32)
                nc.vector.tensor_tensor(den[:], onep[:], rt[:], op=MUL)
                dinv = sm.tile([B, N], f32)
                nc.vector.reciprocal(dinv[:], den[:])
                scl = sm.tile([B, N], f32)
                nc.vector.tensor_tensor(scl[:], ns2[:], dinv[:], op=MUL)
                nc.vector.tensor_tensor(
                    v_sb[:].rearrange("b (n d) -> b n d", d=D), s_sb[:],
                    scl[:, :, None].broadcast_to([B, N, D]),
                    op=MUL)

                if r < ROUTINGS - 1:
                    nc.vector.tensor_copy(v_small[:], v_sb[:])
                    for rr in range(GRP):
                        nc.sync.dma_start(
                            v_rep[32 * rr:32 * rr + 32, :, :],
                            v_small[:].rearrange("b (n d) -> b n d", d=D))

            nc.sync.dma_start(v_d[:], v_sb[:])

    nc.compile()
    return nc


def kernel(x: np.ndarray, W: np.ndarray) -> np.ndarray:
    import ml_dtypes
    from concourse.bass_utils import run_bass_kernel_spmd

    nc = _build()

    bf = ml_dtypes.bfloat16
    xt = np.ascontiguousarray(x.transpose(2, 1, 0)).astype(bf)          # [I,J,B]
    wt = np.ascontiguousarray(W.transpose(3, 1, 0, 2).reshape(I, J, ND)).astype(bf)
    ones4 = np.tile(np.eye(B, dtype=np.float32), (GRP, 1)).astype(bf)

    in_maps = []
    for k in range(NCORES):
        jlo, jhi = k * JC, (k + 1) * JC
        in_maps.append({
            "xt": np.ascontiguousarray(xt[:, jlo:jhi, :]).reshape(I, JC * B),
            "wt": np.ascontiguousarray(wt[:, jlo:jhi, :]),
            "ones4": ones4,
        })

    res = run_bass_kernel_spmd(nc, in_maps, list(range(NCORES)))
    v = np.asarray(res.results[0]["v"], dtype=np.float32)
    return v.reshape(B, N, D)


if __name__ == "__main__":
    rng = np.random.default_rng(0)
    x = rng.normal(size=(B, J, I)).astype(np.float32)
    W = rng.normal(size=(N, J, D, I)).astype(np.float32) * 0.05
    v = kernel(x, W)
    print(v.shape, v.dtype, np.abs(v).max())


# revision 9
# speedup vs baseline: 1.1890x; 1.1890x over previous
"""CapsuleLayer dynamic-routing kernel for 8 Trainium2 NeuronCores.

Math (reference):
    u_hat[b,n,j,d] = sum_i W[n,j,d,i] * x[b,j,i]
    b = 0; for r in 0..2:
        c = softmax_n(b); s[b,n,d] = sum_j c*u_hat; v = squash_d(s)
        if r < 2: b += sum_d v*u_hat
    return v  [B, N, D]

Sharding: J (input capsules, 2048) split 8 ways -> Jc=256 per core.
Softmax over n is local; only s needs a 256 KiB AllReduce per iteration.

Per-core dataflow, one sweep over W per routing iteration (u_hat is
recomputed from SBUF-streamed W each iteration; never materialized):
  - j processed in groups of 4: 4 PE sub-matmuls via column tiling
    (tile_position=(0,32r)) produce u_hat group tile
    [128=(4j x 32b), (n,d)] in PSUM.
  - r0: softmax(0) is uniform, so u_hat is accumulated over all j
    directly in PSUM; s0 = (1/N) * strip-sum. No vector work at all.
  - r>=1: logits[p=(j,b), (g,n)] += sum_d v_{r-1}*u_hat  (DVE mult +
    segmented reduce over d); softmax over n is local to each
    (partition, group) -> c; tmp2 = c (x) u_hat on GpSimd.
  - s accumulated over j by a PE matmul with a stacked-identity lhsT
    (sums the 4 j-strips per b), accumulating across groups in PSUM.
    s-matmuls are emitted one group late so they don't block the next
    group's u_hat matmuls in the in-order PE queue.
  - AllReduce s across cores, squash redundantly on every core.
"""

import functools
import numpy as np

B, J, I = 32, 2048, 16
N, D = 64, 32
NCORES = 8
JC = J // NCORES          # 256 j per core
GRP = 4                   # j's per group (PE column strips)
NG = JC // GRP            # 64 groups
ND = N * D                # 2048
HALF = ND // 2            # 1024 free-dim half (PSUM sizing)
NH = N // 2               # 32 n per half
ROUTINGS = 3
EPS = 1e-7


@functools.lru_cache(maxsize=1)
def _build():
    import concourse.bass as bass
    import concourse.mybir as mybir
    import concourse.bacc as bacc
    import concourse.tile as tile

    f32 = mybir.dt.float32
    bf16 = mybir.dt.bfloat16
    MUL = mybir.AluOpType.mult
    ADD = mybir.AluOpType.add
    AX = mybir.AxisListType.X
    AF = mybir.ActivationFunctionType

    nc = bacc.Bacc("TRN2", target_bir_lowering=False, debug=False,
                   num_devices=NCORES)

    xt_d = nc.dram_tensor("xt", [I, JC * B], bf16, kind="ExternalInput")
    wt_d = nc.dram_tensor("wt", [I, JC, ND], bf16, kind="ExternalInput")
    ones_d = nc.dram_tensor("ones4", [GRP * B, B], bf16, kind="ExternalInput")
    v_d = nc.dram_tensor("v", [B, ND], f32, kind="ExternalOutput")

    with tile.TileContext(nc) as tc:
        with (
            tc.tile_pool(name="persist", bufs=1) as pp,
            tc.tile_pool(name="wstream", bufs=4) as wp,
            tc.tile_pool(name="work", bufs=4) as wk,
            tc.tile_pool(name="small", bufs=6) as sm,
            tc.tile_pool(name="ups", bufs=3, space="PSUM") as ups_pool,
            tc.tile_pool(name="sps", bufs=1, space="PSUM") as sps_pool,
            tc.tile_pool(name="dram", bufs=1, space="DRAM") as dr,
        ):
            xt = pp.tile([I, JC * B], bf16)
            nc.sync.dma_start(xt[:], xt_d[:])
            ones4 = pp.tile([GRP * B, B], bf16)
            nc.sync.dma_start(ones4[:], ones_d[:])

            logits = pp.tile([128, NG, N], bf16)
            v_rep = pp.tile([128, N, D], bf16)
            v_small = pp.tile([B, ND], bf16)
            s_sb = pp.tile([128, 512], f32)
            v_sb = pp.tile([B, ND], f32)

            cc_in = dr.tile([128, 512], f32)
            cc_out = dr.tile([128, 512], f32)

            def u_mms(u_ps, w_t, g, h, start, stop):
                """16 col-tiled matmuls for one (group, half); rr-outer so
                consecutive chunk matmuls share the stationary lhsT."""
                for rr in range(GRP):
                    j = g * GRP + rr
                    for cch in range(2):
                        nc.tensor.matmul(
                            u_ps[32 * rr:32 * rr + 32,
                                 cch * 512:(cch + 1) * 512],
                            xt[:, j * B:(j + 1) * B],
                            w_t[:, rr, h * HALF + cch * 512:
                                h * HALF + (cch + 1) * 512],
                            start=start, stop=stop,
                            tile_position=(0, 32 * rr),
                            skip_group_check=True,
                        )

            for r in range(ROUTINGS):
                s_ps = sps_pool.tile([128, 512], f32)

                if r == 0:
                    # -- r0: c is uniform; accumulate u_hat over j in PSUM --
                    acc = [ups_pool.tile([128, HALF], f32, name=f"acc{_h}", tag="u_ps") for _h in range(2)]
                    for g in range(NG):
                        w_t = wp.tile([I, GRP, ND], bf16)
                        nc.sync.dma_start(
                            w_t[:], wt_d[:, g * GRP:(g + 1) * GRP, :])
                        for h in range(2):
                            u_mms(acc[h], w_t, g, h,
                                  start=(g == 0), stop=(g == NG - 1))
                    # evac to bf16 SBUF, then strip-sum via ones4 matmul
                    for h in range(2):
                        a_sb = wk.tile([128, HALF], bf16)
                        nc.scalar.activation(a_sb[:], acc[h][:], AF.Copy)
                        for cch in range(2):
                            q = 2 * h + cch
                            nc.tensor.matmul(
                                s_ps[32 * q:32 * q + 32, :],
                                ones4[:],
                                a_sb[:, cch * 512:(cch + 1) * 512],
                                start=True, stop=True,
                                tile_position=(0, 32 * q),
                                skip_group_check=True,
                            )
                else:
                    # -- r>=1: fused logits update + local softmax + s --
                    pending_smm = []
                    for g in range(NG):
                        w_t = wp.tile([I, GRP, ND], bf16)
                        nc.sync.dma_start(
                            w_t[:], wt_d[:, g * GRP:(g + 1) * GRP, :])

                        c_t = sm.tile([128, N], bf16)
                        zrec = sm.tile([128, 1], f32)

                        u_sb_halves = []
                        for h in range(2):
                            u_ps = ups_pool.tile([128, HALF], f32)
                            u_mms(u_ps, w_t, g, h, start=True, stop=True)
                            # emit previous group's s-matmuls behind this
                            # group's u-matmuls in the PE stream
                            if pending_smm:
                                pending_smm.pop(0)()

                            u_sb = wk.tile([128, NH, D], bf16)
                            nc.scalar.activation(u_sb[:], u_ps[:], AF.Copy)
                            tl = wk.tile([128, NH, D], bf16)
                            nc.vector.tensor_tensor(
                                tl[:], u_sb[:],
                                v_rep[:, h * NH:(h + 1) * NH, :], op=MUL)
                            with nc.allow_low_precision("bf16 routing logits"):
                                if r == 1:
                                    nc.vector.tensor_reduce(
                                        logits[:, g, h * NH:(h + 1) * NH],
                                        tl[:], axis=AX, op=ADD)
                                else:
                                    dtmp = sm.tile([128, NH], bf16)
                                    nc.vector.tensor_reduce(
                                        dtmp[:], tl[:], axis=AX, op=ADD)
                                    nc.vector.tensor_add(
                                        logits[:, g, h * NH:(h + 1) * NH],
                                        logits[:, g, h * NH:(h + 1) * NH],
                                        dtmp[:])
                            u_sb_halves.append(u_sb)

                        # local softmax over n for this group's 4 j's
                        e_t = sm.tile([128, N], f32)
                        nc.scalar.activation(e_t[:], logits[:, g, :], AF.Exp)
                        zsum = sm.tile([128, 1], f32)
                        nc.vector.tensor_reduce(zsum[:], e_t[:], axis=AX, op=ADD)
                        nc.vector.reciprocal(zrec[:], zsum[:])
                        nc.vector.tensor_scalar_mul(c_t[:], e_t[:], zrec[:])

                        # tmp2 = c (x) u_hat on GpSimd (c broadcast over d)
                        t2s = []
                        for h in range(2):
                            t2 = wk.tile([128, NH, D], bf16, name="t2", tag="t2")
                            eng = nc.vector if h == 0 else nc.gpsimd
                            eng.tensor_tensor(
                                t2[:], u_sb_halves[h][:],
                                c_t[:, h * NH:(h + 1) * NH, None]
                                .broadcast_to([128, NH, D]),
                                op=MUL)
                            t2s.append(t2)

                        def make_smm(t2s=t2s, g=g):
                            def emit():
                                for h in range(2):
                                    t2f = t2s[h][:].rearrange("p a b -> p (a b)")
                                    for cch in range(2):
                                        q = 2 * h + cch
                                        nc.tensor.matmul(
                                            s_ps[32 * q:32 * q + 32, :],
                                            ones4[:],
                                            t2f[:, cch * 512:(cch + 1) * 512],
                                            start=(g == 0), stop=(g == NG - 1),
                                            tile_position=(0, 32 * q),
                                            skip_group_check=True,
                                        )
                            return emit
                        pending_smm.append(make_smm())
                    while pending_smm:
                        pending_smm.pop(0)()

                # ---- end of sweep: AllReduce s, squash, update v ----
                # everything below stays in the (quarter, b)-strip layout:
                # partition 32q+b holds n in [16q,16q+16), all of d.
                s_evac = sm.tile([128, 512], f32)
                if r == 0:
                    nc.vector.tensor_scalar_mul(s_evac[:], s_ps[:], 1.0 / N)
                else:
                    nc.vector.tensor_copy(s_evac[:], s_ps[:])
                nc.sync.dma_start(cc_in[:], s_evac[:])
                nc.gpsimd.collective_compute(
                    "AllReduce", ADD,
                    replica_groups=[list(range(NCORES))],
                    ins=[cc_in[:].opt()], outs=[cc_out[:].opt()],
                )
                nc.sync.dma_start(s_sb[:], cc_out[:])

                sq = sm.tile([128, 16, D], f32)
                s3 = s_sb[:].rearrange("p (n d) -> p n d", d=D)
                nc.vector.tensor_tensor(sq[:], s3, s3, op=MUL)
                ns2 = sm.tile([128, 16], f32)
                nc.vector.tensor_reduce(ns2[:], sq[:], axis=AX, op=ADD)
                onep = sm.tile([128, 16], f32)
                nc.vector.tensor_scalar_add(onep[:], ns2[:], 1.0)
                rt = sm.tile([128, 16], f32)
                eps_t = sm.tile([128, 1], f32)
                nc.vector.memset(eps_t[:], EPS)
                nc.scalar.activation(rt[:], ns2[:], AF.Sqrt, bias=eps_t[:])
                den = sm.tile([128, 16], f32)
                nc.vector.tensor_tensor(den[:], onep[:], rt[:], op=MUL)
                dinv = sm.tile([128, 16], f32)
                nc.vector.reciprocal(dinv[:], den[:])
                scl = sm.tile([128, 16], f32)
                nc.vector.tensor_tensor(scl[:], ns2[:], dinv[:], op=MUL)
                v4 = sm.tile([128, 16, D], f32)
                nc.vector.tensor_tensor(
                    v4[:], s3,
                    scl[:, :, None].broadcast_to([128, 16, D]),
                    op=MUL)

                if r < ROUTINGS - 1:
                    v4b = sm.tile([128, 512], bf16)
                    nc.vector.tensor_copy(
                        v4b[:], v4[:].rearrange("p a b -> p (a b)"))
                    for q in range(4):
                        nc.sync.dma_start(
                            v_small[:, q * 512:(q + 1) * 512],
                            v4b[32 * q:32 * q + 32, :])
                    for rr in range(GRP):
                        nc.sync.dma_start(
                            v_rep[32 * rr:32 * rr + 32, :, :],
                            v_small[:].rearrange("b (n d) -> b n d", d=D))
                else:
                    for q in range(4):
                        nc.sync.dma_start(
                            v_sb[:, q * 512:(q + 1) * 512],
                            v4[32 * q:32 * q + 32, :])

            nc.sync.dma_start(v_d[:], v_sb[:])

    nc.compile()
    return nc


def kernel(x: np.ndarray, W: np.ndarray) -> np.ndarray:
    import ml_dtypes
    from concourse.bass_utils import run_bass_kernel_spmd

    nc = _build()

    bf = ml_dtypes.bfloat16
    xt = np.ascontiguousarray(x.transpose(2, 1, 0)).astype(bf)          # [I,J,B]
    wt = np.ascontiguousarray(W.transpose(3, 1, 0, 2).reshape(I, J, ND)).astype(bf)
    ones4 = np.tile(np.eye(B, dtype=np.float32), (GRP, 1)).astype(bf)

    in_maps = []
    for k in range(NCORES):
        jlo, jhi = k * JC, (k + 1) * JC
        in_maps.append({
            "xt": np.ascontiguousarray(xt[:, jlo:jhi, :]).reshape(I, JC * B),
            "wt": np.ascontiguousarray(wt[:, jlo:jhi, :]),
            "ones4": ones4,
        })

    res = run_bass_kernel_spmd(nc, in_maps, list(range(NCORES)))
    v = np.asarray(res.results[0]["v"], dtype=np.float32)
    return v.reshape(B, N, D)


if __name__ == "__main__":
    rng = np.random.default_rng(0)
    x = rng.normal(size=(B, J, I)).astype(np.float32)
    W = rng.normal(size=(N, J, D, I)).astype(np.float32) * 0.05
    v = kernel(x, W)
    print(v.shape, v.dtype, np.abs(v).max())
